# revision 1
# baseline (speedup 1.0000x reference)
"""GNN message-passing kernel for Trainium2 (8 NeuronCores, SPMD).

Computes: out = segment_sum((x @ W)[src], dst) + bias
        = segment_sum(x[src], dst) @ W + bias          (linearity)

Sharding: dst nodes split across 8 cores (12500 each). Each core:
  - gathers x[src] rows (bf16) for its edges via dma_gather, with the
    source table split into 4 buckets of <=32768 rows (int16 gather idx)
  - segment-sums on-chip via one-hot matmul: for each 128-edge chunk,
    PSUM[feat, node] += G[edge, feat].T @ S[edge, node], where
    S[e, m] = (rowid[e] == m) is generated on DVE from per-edge row ids
  - applies W + bias on the aggregate, writes out transposed [128, 12500]
Host re-assembles the full [100000, 128] output.
"""
import sys
sys.path.insert(0, "/opt/trn_rl_repo")

import numpy as np
import ml_dtypes

import concourse.bacc as bacc
import concourse.mybir as mybir
import concourse.tile as tile
from concourse.bass_utils import run_bass_kernel_spmd
from concourse.library_config import mlp

N_CORES = 8
GROUP = 128     # dst nodes per psum column block
SLAB_G = 8      # groups per slab
NB = 4          # source buckets (table rows per bucket must fit int16)
D = 128
MAX_GATHER = 8192   # HW limit: dma_gather descriptor ring capacity


def _ceil(a, b):
    return -(-a // b)


def build_layout(edge_index, n_nodes):
    """Vectorized host-side edge partitioning. Returns static layout + per-core
    padded idx/rowid arrays."""
    src = np.asarray(edge_index[0], dtype=np.int64)
    dst = np.asarray(edge_index[1], dtype=np.int64)
    E = src.shape[0]
    npc = n_nodes // N_CORES                    # nodes per core
    NG = _ceil(npc, GROUP)                      # groups per core
    NS = _ceil(NG, SLAB_G)                      # slabs per core
    bucket_rows = _ceil(n_nodes, NB)            # rows per source bucket
    assert bucket_rows <= 32767

    core = dst // npc
    np.minimum(core, N_CORES - 1, out=core)     # guard (n_nodes % N_CORES == 0 here)
    dloc = dst - core * npc
    g = dloc // GROUP
    m = dloc % GROUP
    b = src // bucket_rows
    i16 = (src - b * bucket_rows).astype(np.int16)

    key = (core * NG + g) * NB + b
    order = np.argsort(key, kind="stable")
    ks = key[order]
    counts = np.bincount(key, minlength=N_CORES * NG * NB).reshape(N_CORES, NG, NB)
    caps = np.maximum(128, _ceil(counts.max(axis=0), 128) * 128).astype(np.int64)  # [NG, NB]

    # rank of each edge within its (core, g, b) run
    run_starts = np.zeros(N_CORES * NG * NB, np.int64)
    run_starts[1:] = np.cumsum(counts.reshape(-1))[:-1]
    rank = np.empty(E, np.int64)
    rank[order] = np.arange(E) - run_starts[ks]

    # global padded layout: for s: for b: for g in slab -> block of caps[g, b]
    pad_base = np.zeros((NG, NB), np.int64)
    seg_off = np.zeros((NS, NB), np.int64)
    seg_len = np.zeros((NS, NB), np.int64)
    off = 0
    for s in range(NS):
        gs = range(s * SLAB_G, min((s + 1) * SLAB_G, NG))
        for bb in range(NB):
            seg_off[s, bb] = off
            for gg in gs:
                pad_base[gg, bb] = off
                off += caps[gg, bb]
            seg_len[s, bb] = off - seg_off[s, bb]
    total = off

    pos = pad_base[g, b] + rank                 # per-edge slot in padded layout
    s_of_e = g // SLAB_G
    q = pos - seg_off[s_of_e, b]                # seg-relative slot
    col16 = (seg_off[s_of_e, b] // 16) + q // 16
    row16 = q % 16

    idx_w = np.zeros((N_CORES, 16, total // 16), np.int16)
    rid_w = np.full((N_CORES, 128, total // 128), -1.0, np.float32)
    idx_w[core, row16, col16] = i16
    rid_w[core, pos % 128, pos // 128] = m.astype(np.float32)
    idx_w = np.tile(idx_w, (1, 8, 1))           # replicate for the 8 Q7 cores
    rid_w = rid_w.astype(ml_dtypes.bfloat16)

    return dict(npc=npc, NG=NG, NS=NS, bucket_rows=bucket_rows,
                caps=caps, seg_off=seg_off, seg_len=seg_len, pad_base=pad_base,
                total=total, idx_w=idx_w, rid_w=rid_w)


def build_program(lay, n_nodes, reps=1, parts=("gather", "onehot", "mm1", "mm2"),
                  single_packet=False):
    parts = frozenset(parts)
    npc, NG, NS = lay["npc"], lay["NG"], lay["NS"]
    caps, seg_off, seg_len, pad_base = (lay["caps"], lay["seg_off"],
                                        lay["seg_len"], lay["pad_base"])
    total = lay["total"]
    bucket_rows = lay["bucket_rows"]

    nc = bacc.Bacc("TRN2", target_bir_lowering=False, debug=False,
                   enable_asserts=False, num_swdge_queues=4)
    xbf = nc.dram_tensor("xbf", [n_nodes, D], mybir.dt.bfloat16, kind="ExternalInput")
    idx = nc.dram_tensor("idx", [128, total // 16], mybir.dt.int16, kind="ExternalInput")
    rid = nc.dram_tensor("rid", [128, total // 128], mybir.dt.bfloat16, kind="ExternalInput")
    iota = nc.dram_tensor("iota", [128, GROUP], mybir.dt.bfloat16, kind="ExternalInput")
    w = nc.dram_tensor("w", [D, D], mybir.dt.float32, kind="ExternalInput")
    bias = nc.dram_tensor("bias", [D, 1], mybir.dt.float32, kind="ExternalInput")
    outT = nc.dram_tensor("outT", [D, npc], mybir.dt.float32, kind="ExternalOutput")

    with tile.TileContext(nc) as tc:
        with (
            tc.tile_pool(name="const", bufs=1) as cpool,
            tc.tile_pool(name="g", bufs=5) as gpool,
            tc.tile_pool(name="s", bufs=4) as spool,
            tc.tile_pool(name="a", bufs=3) as apool,
            tc.tile_pool(name="o", bufs=3) as opool,
            tc.tile_pool(name="ps", bufs=3, space="PSUM") as pspool,
            tc.tile_pool(name="p2", bufs=2, space="PSUM") as p2pool,
        ):
            idx_t = cpool.tile([128, total // 16], mybir.dt.int16)
            nc.sync.dma_start(idx_t[:], idx.ap())
            rid_t = cpool.tile([128, total // 128], mybir.dt.bfloat16)
            nc.sync.dma_start(rid_t[:], rid.ap())
            iota_t = cpool.tile([128, GROUP], mybir.dt.bfloat16)
            nc.sync.dma_start(iota_t[:], iota.ap())
            w_t = cpool.tile([D, D], mybir.dt.float32)
            nc.sync.dma_start(w_t[:], w.ap())
            bias_t = cpool.tile([D, 1], mybir.dt.float32)
            nc.sync.dma_start(bias_t[:], bias.ap())

            nc.gpsimd.load_library(mlp)

            for _rep in range(reps):
              for s in range(NS):
                  gs = list(range(s * SLAB_G, min((s + 1) * SLAB_G, NG)))
                  gts, sts = [], []
                  for b in range(NB):
                      sl = int(seg_len[s, b])
                      nch = sl // 128
                      o16 = int(seg_off[s, b]) // 16
                      och = int(seg_off[s, b]) // 128
                      gt = gpool.tile([128, nch, D], mybir.dt.bfloat16, tag="g")
                      if "gather" in parts:
                          for goff in range(0, sl, MAX_GATHER):
                              n_i = min(MAX_GATHER, sl - goff)
                              nc.gpsimd.dma_gather(
                                  gt[:, goff // 128:(goff + n_i) // 128, :],
                                  xbf.ap()[b * bucket_rows:(b + 1) * bucket_rows, :],
                                  idx_t[:, o16 + goff // 16:o16 + (goff + n_i) // 16],
                                  n_i, n_i, D,
                                  single_packet=single_packet,
                                  queue_num=b,
                              )
                      st = spool.tile([128, nch, GROUP], mybir.dt.bfloat16, tag="s")
                      if "onehot" in parts:
                          nc.vector.tensor_tensor(
                              st[:],
                              rid_t[:, och:och + nch].unsqueeze(2).broadcast_to([128, nch, GROUP]),
                              iota_t[:].unsqueeze(1).broadcast_to([128, nch, GROUP]),
                              mybir.AluOpType.is_equal,
                          )
                      gts.append(gt)
                      sts.append(st)

                  pt = pspool.tile([128, len(gs) * GROUP], mybir.dt.float32, tag="ps")
                  if "mm1" in parts:
                    for gi, gg in enumerate(gs):
                      nchunks = [int(caps[gg, b]) // 128 for b in range(NB)]
                      first = True
                      for b in range(NB):
                          base = (int(pad_base[gg, b]) - int(seg_off[s, b])) // 128
                          for i in range(nchunks[b]):
                              col = base + i
                              nc.tensor.matmul(
                                  pt[:, gi * GROUP:(gi + 1) * GROUP],
                                  gts[b][:, col, :],
                                  sts[b][:, col, :],
                                  start=first,
                                  stop=(b == NB - 1 and i == nchunks[b] - 1),
                              )
                              first = False

                  at = apool.tile([128, len(gs) * GROUP], mybir.dt.float32, tag="a")
                  nc.vector.tensor_copy(at[:], pt[:])

                  n0 = s * SLAB_G * GROUP
                  nodes_s = min(npc - n0, SLAB_G * GROUP)
                  for j0 in range(0, nodes_s, 512):
                      nj = min(512, nodes_s - j0)
                      p2 = p2pool.tile([128, nj], mybir.dt.float32, tag="p2")
                      if "mm2" in parts:
                          nc.tensor.matmul(p2[:], w_t[:], at[:, j0:j0 + nj],
                                           start=True, stop=True)
                      ot = opool.tile([128, nj], mybir.dt.float32, tag="o")
                      nc.scalar.activation(ot[:], p2[:],
                                           mybir.ActivationFunctionType.Identity,
                                           bias=bias_t[:], scale=1.0)
                      nc.sync.dma_start(outT.ap()[:, n0 + j0:n0 + j0 + nj], ot[:])

    nc.compile()
    return nc


def prepare(x, edge_index, weight, bias):
    """Build layout + program + per-core input maps. Returns
    (nc, in_maps, assemble) where assemble(results) -> full output."""
    x = np.asarray(x, dtype=np.float32)
    weight = np.asarray(weight, dtype=np.float32)
    bias = np.asarray(bias, dtype=np.float32)
    n_nodes = x.shape[0]
    lay = build_layout(edge_index, n_nodes)
    nc = build_program(lay, n_nodes)

    xbf = np.ascontiguousarray(x.astype(ml_dtypes.bfloat16))
    iota_np = np.ascontiguousarray(
        np.broadcast_to(np.arange(GROUP, dtype=np.float32), (128, GROUP))
    ).astype(ml_dtypes.bfloat16)
    w_np = np.ascontiguousarray(weight)
    bias_np = np.ascontiguousarray(bias.reshape(D, 1))

    in_maps = []
    for c in range(N_CORES):
        in_maps.append({
            "xbf": xbf,
            "idx": np.ascontiguousarray(lay["idx_w"][c]),
            "rid": np.ascontiguousarray(lay["rid_w"][c]),
            "iota": iota_np,
            "w": w_np,
            "bias": bias_np,
        })

    npc = lay["npc"]

    def assemble(results):
        out = np.empty((n_nodes, D), np.float32)
        for c in range(N_CORES):
            out[c * npc:(c + 1) * npc] = results[c]["outT"].T
        return out

    return nc, in_maps, assemble


def kernel(x, edge_index, weight, bias):
    nc, in_maps, assemble = prepare(x, edge_index, weight, bias)
    res = run_bass_kernel_spmd(nc, in_maps, core_ids=list(range(N_CORES)))
    return assemble(res.results)

